# revision 5
# baseline (speedup 1.0000x reference)
"""Trainium2 Bass kernel for 4-bit-quantized Linear: y = x @ dequant(Wq4).T + bias.

Sharding: tensor-parallel over out_features (11008 rows -> 8 cores x 1408,
last core zero-padded), x replicated (fed pre-transposed fp16), outputs
concatenated on host.

Per-core device kernel:
  - dequant int4 (packed 2-nibbles-per-int32) -> fp16 weights, scaled by
    per-block norm:  W = (2*q - 15) * (norm/15)
  - PE-transpose dequantized [o,k] tiles into K-major [k,o] layout
  - fp16 matmul (PSUM fp32 accumulation over K=4096) + bias add
Output columns are processed in 3 chunks (512/512/384) so chunk c+1's
dequant overlaps chunk c's matmuls.
"""
import os
import numpy as np

import concourse.bass as bass
import concourse.bacc as bacc
import concourse.mybir as mybir
import concourse.tile as tile
from concourse.bass_utils import run_bass_kernel_spmd

F16, F32, I32 = mybir.dt.float16, mybir.dt.float32, mybir.dt.int32

# Problem constants (hardcoded per contract)
TOKENS, IN, OUT = 4096, 4096, 11008
GROUP, BLOCKS, HALF = 16, 256, 8
N_CORES = 8
O_C = 1408                      # padded per-core out rows (11 tiles of 128)
KT = IN // 128                  # 32 k-slabs
TC = 512                        # t super-chunk
O_CHUNKS = [(0, 512), (512, 512), (1024, 384)]   # (offset, width); 128-tile aligned


def build_bass(tokens=TOKENS, in_=IN, o_c=O_C, tc_sz=TC, o_chunks=None):
    """Build the per-core Bass program (parameterized for small-scale sim tests)."""
    kt = in_ // 128
    blocks = in_ // GROUP
    if o_chunks is None:
        o_chunks = O_CHUNKS
    max_w = max(w for _, w in o_chunks)
    n_tc = tokens // tc_sz
    tl_per_tc = tc_sz // 128

    nc = bacc.Bacc("TRN2", target_bir_lowering=False, debug=False)

    xt_d = nc.dram_tensor("xt", [in_, tokens], F16, kind="ExternalInput")
    wq_d = nc.dram_tensor("wq", [o_c, blocks * HALF], I32, kind="ExternalInput")
    wn_d = nc.dram_tensor("wn", [o_c, blocks], F16, kind="ExternalInput")
    br_d = nc.dram_tensor("bias_rep", [128, o_c], F32, kind="ExternalInput")
    id_d = nc.dram_tensor("ident", [128, 128], F16, kind="ExternalInput")
    y_d = nc.dram_tensor("y", [tokens, o_c], F32, kind="ExternalOutput")

    with tile.TileContext(nc) as tc:
        with (
            tc.tile_pool(name="const", bufs=1) as cst,
            tc.tile_pool(name="dq", bufs=1) as dq,
            tc.tile_pool(name="dqv", bufs=2) as dqv,
            tc.tile_pool(name="wt", bufs=2) as wtp,
            tc.tile_pool(name="xp", bufs=2) as xp,
            tc.tile_pool(name="yp", bufs=2) as yp,
            tc.tile_pool(name="pst", bufs=2, space=bass.MemorySpace.PSUM) as pst,
            tc.tile_pool(name="psm", bufs=3, space=bass.MemorySpace.PSUM) as psm,
        ):
            ident = cst.tile([128, 128], F16, tag="ident")
            nc.gpsimd.dma_start(ident[:], id_d[:])
            bias_sb = cst.tile([128, o_c], F32, tag="bias")
            nc.gpsimd.dma_start(bias_sb[:], br_d[:])

            for oc_i, (o_off, o_w) in enumerate(o_chunks):
                n_ot = o_w // 128
                # ---------------- dequant this chunk's o-tiles ----------------
                wtc = wtp.tile([128, kt, max_w], F16, tag="wtc")
                for oti in range(n_ot):
                    ot = o_off // 128 + oti
                    v = dqv.tile([128, blocks, HALF], I32, tag="v")
                    nc.gpsimd.dma_start(
                        v[:], wq_d[ot * 128:(ot + 1) * 128].rearrange(
                            "o (b h) -> o b h", h=HALF))
                    nrm = dqv.tile([128, blocks], F16, tag="nrm")
                    nc.gpsimd.dma_start(nrm[:], wn_d[ot * 128:(ot + 1) * 128])
                    s = dq.tile([128, blocks], F32, tag="s")
                    nc.vector.tensor_scalar_mul(s[:], nrm[:], 1.0 / 15.0)

                    a = dq.tile([128, blocks, HALF], I32, tag="a")
                    b = dq.tile([128, blocks, HALF], I32, tag="b")
                    zq = dq.tile([128, blocks, GROUP], F16, tag="zq")
                    # lo nibble -> even g, hi nibble -> odd g; z = 2*q - 15
                    nc.vector.tensor_scalar(
                        a[:], v[:], 15, None, mybir.AluOpType.bitwise_and)
                    nc.vector.tensor_scalar(
                        b[:], v[:], 4, None, mybir.AluOpType.logical_shift_right)
                    nc.scalar.activation(
                        zq[:, :, 0::2], a[:],
                        mybir.ActivationFunctionType.Copy, bias=-15.0, scale=2.0)
                    nc.scalar.activation(
                        zq[:, :, 1::2], b[:],
                        mybir.ActivationFunctionType.Copy, bias=-15.0, scale=2.0)
                    # W = z * (norm/15), broadcast norm over the group dim
                    s_b = bass.AP(s[:].tensor, s[:].offset, s[:].ap + [[0, GROUP]])
                    nc.vector.tensor_tensor(
                        zq[:], zq[:], s_b, mybir.AluOpType.mult)

                    # transpose [o,k] -> [k,o] via PE, up to 4 tiles per PSUM bank
                    tb = min(4, kt)
                    for c4 in range((kt + tb - 1) // tb):
                        pt = pst.tile([128, tb, 128], F16, tag="pt")
                        ks = [c4 * tb + j for j in range(tb) if c4 * tb + j < kt]
                        for j, k in enumerate(ks):
                            nc.tensor.transpose(
                                pt[:, j, :], zq[:, k * 8:(k + 1) * 8, :], ident[:])
                        # one strided copy drains the whole bank: dest strided over k
                        dst = bass.AP(
                            wtc[:].tensor, wtc[:].offset
                            + ks[0] * max_w + oti * 128,
                            [wtc[:].ap[0], [max_w, len(ks)], [1, 128]])
                        nc.scalar.copy(dst, pt[:, :len(ks), :])

                # ---------------- matmul for this chunk ----------------
                for tci in range(n_tc):
                    xtt = xp.tile([128, kt, tc_sz], F16, tag="xtt")
                    nc.gpsimd.dma_start(
                        xtt[:],
                        xt_d.ap().rearrange("(s p) t -> p s t", p=128)
                        [:, :, tci * tc_sz:(tci + 1) * tc_sz])
                    y_sb = yp.tile([128, tl_per_tc, max_w], F32, tag="y")
                    for tl in range(tl_per_tc):
                        ps = psm.tile([128, max_w], F32, tag="ps")
                        for k in range(kt):
                            nc.tensor.matmul(
                                ps[:, :o_w],
                                xtt[:, k, tl * 128:(tl + 1) * 128],
                                wtc[:, k, :o_w],
                                start=(k == 0), stop=(k == kt - 1))
                        nc.vector.tensor_tensor(
                            y_sb[:, tl, :o_w], ps[:, :o_w],
                            bias_sb[:, o_off:o_off + o_w], mybir.AluOpType.add)
                    # one DMA for the whole [tc_sz, o_w] block
                    nc.gpsimd.dma_start(
                        y_d[tci * tc_sz:(tci + 1) * tc_sz, o_off:o_off + o_w]
                        .rearrange("(l p) o -> p l o", p=128),
                        y_sb[:, :, :o_w])
    nc.compile()
    return nc


def _prep_host_inputs(x, weight_q4, weight_norm, bias):
    """Host-side shard + layout prep. Returns in_maps for 8 cores."""
    xt = np.ascontiguousarray(x.T).astype(np.float16)
    o_pad = N_CORES * O_C
    wq = np.zeros((o_pad, BLOCKS * HALF), np.int32)
    wq[:OUT] = weight_q4.reshape(OUT, BLOCKS * HALF)
    wn = np.zeros((o_pad, BLOCKS), np.float16)
    wn[:OUT] = weight_norm.reshape(OUT, BLOCKS).astype(np.float16)
    bs = np.zeros((o_pad,), np.float32)
    bs[:OUT] = bias
    ident = np.eye(128, dtype=np.float16)

    in_maps = []
    for c in range(N_CORES):
        sl = slice(c * O_C, (c + 1) * O_C)
        in_maps.append({
            "xt": xt,
            "wq": np.ascontiguousarray(wq[sl]),
            "wn": np.ascontiguousarray(wn[sl]),
            "bias_rep": np.ascontiguousarray(
                np.broadcast_to(bs[sl][None, :], (128, O_C))),
            "ident": ident,
        })
    return in_maps


_CACHE = {}


def _run(in_maps):
    if "nc" not in _CACHE:
        _CACHE["nc"] = build_bass()
    nc = _CACHE["nc"]
    res = run_bass_kernel_spmd(nc, in_maps, list(range(N_CORES)))
    return res


def kernel(x, weight_q4, weight_norm, bias):
    in_maps = _prep_host_inputs(
        np.asarray(x), np.asarray(weight_q4),
        np.asarray(weight_norm), np.asarray(bias))
    res = _run(in_maps)
    outs = [res.results[c]["y"] for c in range(N_CORES)]
    y = np.concatenate(outs, axis=1)[:, :OUT]
    return np.ascontiguousarray(y.astype(np.float32))
